# revision 7
# baseline (speedup 1.0000x reference)
"""Trainium2 Bass kernel for nn_BDHBlock (pre-LN latent block with
softmax-free attention and sigmoid gating).

Sharding: data-parallel over batch B=16 across 8 cores (2 per core).
No collectives; outputs are concatenated on the host.

Per-core math (B_loc=2, N=1024, D=768, H=12, HD=64), all matmuls fp16
with fp32 PSUM accumulation:
  xn   = LayerNorm(x) * ln_w + ln_b              (token-major, fp32)
  lat  = relu(xn @ enc_w.T + enc_b)              (feature-major)
  qk   = rope(lat @ qk_w.T + qk_b) / sqrt(sqrt(HD))   (token-major)
  v    = lat @ v_w.T + v_b                       (token-major)
  T_h  = qk_h^T @ v_h         per (b,h)          [HD, HD]
  attn_h = qk_h @ T_h      (== (qk qk^T/8) v by associativity)
  out  = x + sigmoid(xn @ gate_w.T + gate_b) * (attn @ out_w.T + out_b)

The softmax-free attention makes scores@v associative, so the N x N
score matrices are never materialized.
"""

import os
import sys

for _p in ("/opt/trn_rl_repo", "/root/.axon_site/_ro/trn_rl_repo"):
    if os.path.isdir(_p) and _p not in sys.path:
        sys.path.insert(0, _p)

import math
import numpy as np

import concourse.bass as bass
import concourse.mybir as mybir
from concourse import bacc
from concourse import bass_utils
from concourse.bass import ts, ds
from concourse.tile import TileContext

F32 = mybir.dt.float32
F16 = mybir.dt.float16
AF = mybir.ActivationFunctionType

P = 128          # partitions
D = 768
KT = D // P      # 6 d-tiles
B_LOC = 2        # batch elements per core
SEQ = 1024
T = B_LOC * SEQ  # 2048 tokens per core
NT = T // P      # 16 token tiles
TPB = SEQ // P   # 8 token tiles per batch element
TW = 512         # token window (feature-major matmul free dim)
NTW = T // TW    # 4
JW = 384         # feature window (token-major matmul free dim)
NJW = D // JW    # 2
H = 12
HD = 64
EPS = 1e-5
QK_SCALE = 1.0 / math.sqrt(math.sqrt(HD))  # applied twice => 1/sqrt(HD)

# weight prep order: gate_w reuses enc_w's slot (enc phase is done by then)
W_NAMES = ["enc_w", "qk_w", "v_w", "out_w", "gate_w"]


def build_nc():
    nc = bacc.Bacc("TRN2", target_bir_lowering=False, debug=False)

    x_in = nc.dram_tensor("x", [B_LOC, SEQ, D], F32, kind="ExternalInput")
    rope_in = nc.dram_tensor("rope_emb", [SEQ, HD], F32, kind="ExternalInput")
    vecs = {}
    for nm in ["ln_w", "ln_b", "enc_b", "qk_b", "v_b", "out_b", "gate_b"]:
        vecs[nm] = nc.dram_tensor(nm, [D], F32, kind="ExternalInput")
    w_in = {nm: nc.dram_tensor(nm, [D, D], F32, kind="ExternalInput")
            for nm in W_NAMES}
    out_t = nc.dram_tensor("out", [B_LOC, SEQ, D], F32, kind="ExternalOutput")

    x_flat = x_in.ap().rearrange("b n d -> (b n) d")
    out_flat = out_t.ap().rearrange("b n d -> (b n) d")

    with TileContext(nc) as tc:
        with (
            tc.tile_pool(name="consts", bufs=1) as cp,
            tc.tile_pool(name="wrot", bufs=4) as wrot,
            tc.tile_pool(name="big", bufs=3) as bigp,
            tc.tile_pool(name="work", bufs=2) as wk,
            tc.tile_pool(name="stats", bufs=4) as stp,
            tc.tile_pool(name="ropewk", bufs=2) as rwk,
            tc.tile_pool(name="tbuf", bufs=12) as tbp,
            tc.tile_pool(name="dram", bufs=2, space="DRAM") as dp,
            tc.tile_pool(name="dram1", bufs=1, space="DRAM") as dp1,
            tc.tile_pool(name="ps512", bufs=3, space="PSUM") as ps512,
            tc.tile_pool(name="ps384", bufs=3, space="PSUM") as ps384,
            tc.tile_pool(name="psT", bufs=2, space="PSUM") as psT,
        ):
            # ---------------- constants / weight prep ----------------
            with nc.named_scope("prep"):
                # rope tables: [128, TPB, 4, 32] = cosE, sinE, sinO, cosO
                rp = wk.tile([P, TPB, HD], F32, tag="ropein")
                nc.sync.dma_start(
                    rp[:], rope_in.ap().rearrange("(t p) d -> p t d", p=P))
                tabs = cp.tile([P, TPB, 4, HD // 2], F32, tag="ropetabs")
                halfpi_t = cp.tile([P, 1], F32, tag="halfpi")
                nc.vector.memset(halfpi_t[:], math.pi / 2.0)
                eps_t = cp.tile([P, 1], F32, tag="epsc")
                nc.vector.memset(eps_t[:], EPS)
                nc.scalar.activation(tabs[:, :, 0, :], rp[:, :, 0::2], AF.Sin,
                                     bias=halfpi_t[:])
                nc.scalar.activation(tabs[:, :, 1, :], rp[:, :, 0::2], AF.Sin)
                nc.scalar.activation(tabs[:, :, 2, :], rp[:, :, 1::2], AF.Sin)
                nc.scalar.activation(tabs[:, :, 3, :], rp[:, :, 1::2], AF.Sin,
                                     bias=halfpi_t[:])
                nc.vector.tensor_scalar_mul(tabs[:], tabs[:], QK_SCALE)

                # broadcast-to-all-partitions tiles for free-dim vectors
                bc = {}
                for nm in ["ln_w", "ln_b", "qk_b", "v_b", "out_b", "gate_b"]:
                    bc[nm] = cp.tile([P, D], F16, tag=f"bc_{nm}",
                                     name=f"bc_{nm}")
                    nc.gpsimd.dma_start(
                        out=bc[nm][:],
                        in_=vecs[nm].ap()[None, :].to_broadcast((P, D)))
                # enc bias, per-partition layout [128, KT]
                encb = cp.tile([P, KT], F32, tag="encb")
                nc.sync.dma_start(
                    encb[:], vecs["enc_b"].ap().rearrange("(k p) -> p k", p=P))

                # weights: load fp32, cast fp16, roundtrip through DRAM with
                # a transposing read -> W^T laid out [d(part), k, j]
                wT = {}
                for nm in W_NAMES:
                    wT[nm] = wrot.tile([P, KT, D], F16, tag="wT",
                                       name=f"wT_{nm}")
                    w16d = dp.tile([D, D], F16, tag="wdram", name=f"w16_{nm}")
                    for k in range(KT):
                        wld = wk.tile([P, D], F32, tag="wload")
                        nc.sync.dma_start(wld[:], w_in[nm].ap()[ts(k, P), :])
                        wc = wk.tile([P, D], F16, tag="wcast")
                        nc.any.tensor_copy(wc[:], wld[:])
                        nc.sync.dma_start(w16d[ts(k, P), :], wc[:])
                    for k in range(KT):
                        nc.sync.dma_start(wT[nm][:, k, :], w16d[:, ts(k, P)],
                                          transpose=True)

            xn_dram = dp1.tile([T, D], F16, tag="xn_dram")
            qk_dram = dp1.tile([T, D], F16, tag="qk_dram")

            # ---------------- LayerNorm (token-major) ----------------
            with nc.named_scope("ln"):
                for i in range(NT):
                    xt = wk.tile([P, D], F32, tag="xin")
                    nc.sync.dma_start(xt[:], x_flat[ts(i, P), :])
                    xg = xt[:].rearrange("p (s c) -> p s c", c=256)
                    stats = stp.tile([P, 3, 6], F32, tag="bnstats")
                    for s in range(3):
                        nc.vector.bn_stats(stats[:, s, :], xg[:, s, :])
                    mv = stp.tile([P, 2], F32, tag="bnmv")
                    nc.vector.bn_aggr(mv[:], stats[:])
                    rs = stp.tile([P, 1], F32, tag="rstd")
                    nc.scalar.activation(rs[:], mv[:, 1:2], AF.Sqrt,
                                         bias=eps_t[:])
                    nc.vector.reciprocal(rs[:], rs[:])
                    nc.vector.tensor_scalar(
                        xt[:], xt[:], mv[:, 0:1], rs[:],
                        op0=mybir.AluOpType.subtract, op1=mybir.AluOpType.mult)
                    nc.any.tensor_mul(xt[:], xt[:], bc["ln_w"][:])
                    xn16 = wk.tile([P, D], F16, tag="xn16")
                    nc.any.tensor_add(xn16[:], xt[:], bc["ln_b"][:])
                    nc.sync.dma_start(xn_dram[ts(i, P), :], xn16[:])

            # xn^T: feature-major [128, KT, T]; lives until the gate matmuls
            # at the very end, so it gets its own slot outside the rotation.
            xnT = cp.tile([P, KT, T], F16, tag="xnT")
            with nc.named_scope("xnT"):
                for k in range(KT):
                    for h2 in range(2):
                        nc.sync.dma_start(
                            xnT[:, k, ts(h2, SEQ)],
                            xn_dram[ts(h2, SEQ), ts(k, P)], transpose=True)

            # ---------------- encoder: latT = relu(Wenc @ xn^T) ------
            latT = bigp.tile([P, KT, T], F16, tag="big", name="latT")
            with nc.named_scope("enc"):
                for tw in range(NTW):
                    for j in range(KT):
                        ps = ps512.tile([P, TW], F32, tag="ps512")
                        for k in range(KT):
                            nc.tensor.matmul(
                                ps[:], wT["enc_w"][:, k, ts(j, P)],
                                xnT[:, k, ts(tw, TW)],
                                start=(k == 0), stop=(k == KT - 1))
                        nc.scalar.activation(latT[:, j, ts(tw, TW)], ps[:],
                                             AF.Relu, bias=encb[:, j:j + 1])

            # ---------------- qk (token-major) + rope ----------------
            qkR = bigp.tile([P, NT, D], F16, tag="big", name="qkR")
            with nc.named_scope("qk"):
                for i in range(NT):
                    ti = i % TPB
                    for jw in range(NJW):
                        ps = ps384.tile([P, JW], F32, tag="ps384")
                        for k in range(KT):
                            nc.tensor.matmul(
                                ps[:], latT[:, k, ts(i, P)],
                                wT["qk_w"][:, k, ts(jw, JW)],
                                start=(k == 0), stop=(k == KT - 1))
                        xb = rwk.tile([P, JW], F32, tag="ropexb")
                        nc.vector.tensor_add(
                            xb[:], ps[:], bc["qk_b"][:, ts(jw, JW)])
                        # rope on 6 heads at once via step-0 broadcast tables
                        xbh = xb[:].rearrange("p (h d) -> p h d", d=HD)
                        x1 = xbh[:, :, 0::2]
                        x2 = xbh[:, :, 1::2]
                        o = qkR[:, i, ts(jw, JW)].rearrange(
                            "p (h d) -> p h d", d=HD)
                        nh = JW // HD
                        cosE = tabs[:, ti, 0, None, :].to_broadcast(
                            (P, nh, HD // 2))
                        sinE = tabs[:, ti, 1, None, :].to_broadcast(
                            (P, nh, HD // 2))
                        sinO = tabs[:, ti, 2, None, :].to_broadcast(
                            (P, nh, HD // 2))
                        cosO = tabs[:, ti, 3, None, :].to_broadcast(
                            (P, nh, HD // 2))
                        p1 = rwk.tile([P, nh, HD // 2], F32, tag="ropep1")
                        p2 = rwk.tile([P, nh, HD // 2], F32, tag="ropep2")
                        nc.any.tensor_mul(p1[:], x1, cosE)
                        nc.any.tensor_mul(p2[:], x2, sinE)
                        nc.any.tensor_sub(o[:, :, 0:HD // 2], p1[:], p2[:])
                        p3 = rwk.tile([P, nh, HD // 2], F32, tag="ropep1")
                        p4 = rwk.tile([P, nh, HD // 2], F32, tag="ropep2")
                        nc.any.tensor_mul(p3[:], x1, sinO)
                        nc.any.tensor_mul(p4[:], x2, cosO)
                        nc.any.tensor_add(o[:, :, HD // 2:], p3[:], p4[:])
                    nc.sync.dma_start(qk_dram[ts(i, P), :], qkR[:, i, :])

            # ---------------- v (token-major) ------------------------
            vtm = bigp.tile([P, NT, D], F16, tag="big", name="v")
            with nc.named_scope("v"):
                for i in range(NT):
                    for jw in range(NJW):
                        ps = ps384.tile([P, JW], F32, tag="ps384")
                        for k in range(KT):
                            nc.tensor.matmul(
                                ps[:], latT[:, k, ts(i, P)],
                                wT["v_w"][:, k, ts(jw, JW)],
                                start=(k == 0), stop=(k == KT - 1))
                        nc.vector.tensor_add(
                            vtm[:, i, ts(jw, JW)], ps[:],
                            bc["v_b"][:, ts(jw, JW)])

            # qk^T: feature-major [128, KT, T] (takes latT's slot)
            qkT = bigp.tile([P, KT, T], F16, tag="big", name="qkT")
            with nc.named_scope("qkT"):
                for k in range(KT):
                    for h2 in range(2):
                        nc.sync.dma_start(
                            qkT[:, k, ts(h2, SEQ)],
                            qk_dram[ts(h2, SEQ), ts(k, P)], transpose=True)

            # ---------------- attention ------------------------------
            # M1: T_h = qk_h^T @ v_h  [HD, HD] per (b, head); head pairs
            # packed into array column halves.  M2: attnT_h = T_h^T @ qkT_h.
            # All M1 products first so qkR/v are fully released before the
            # attnT slot (which reuses qkR's ring slot) is first written.
            t16s = {}
            with nc.named_scope("attn_m1"):
                for b in range(B_LOC):
                    for hp in range(KT):
                        hA, hB = 2 * hp, 2 * hp + 1
                        pt = psT.tile([P, HD], F32, tag="psT")
                        for m in range(TPB):
                            mt = b * TPB + m
                            nc.tensor.matmul(
                                pt[0:HD, :],
                                qkR[:, mt, ts(hA, HD)], vtm[:, mt, ts(hA, HD)],
                                start=(m == 0), stop=(m == TPB - 1),
                                tile_position=(0, 0))
                            nc.tensor.matmul(
                                pt[HD:P, :],
                                qkR[:, mt, ts(hB, HD)], vtm[:, mt, ts(hB, HD)],
                                start=(m == 0), stop=(m == TPB - 1),
                                tile_position=(0, HD))
                        t16 = tbp.tile([P, HD], F16, tag="t16",
                                       name=f"t16_{b}_{hp}")
                        nc.any.tensor_copy(t16[:], pt[:])
                        t16s[(b, hp)] = t16

            attnT = bigp.tile([P, KT, T], F16, tag="big", name="attnT")
            with nc.named_scope("attn_m2"):
                for b in range(B_LOC):
                    for hp in range(KT):
                        t16 = t16s[(b, hp)]
                        for nw in range(2):
                            col = b * SEQ + nw * TW
                            ps = ps512.tile([P, TW], F32, tag="ps512")
                            nc.tensor.matmul(
                                ps[0:HD, :], t16[0:HD, :],
                                qkT[0:HD, hp, ds(col, TW)],
                                start=True, stop=True, tile_position=(0, 0))
                            nc.tensor.matmul(
                                ps[HD:P, :], t16[HD:P, :],
                                qkT[HD:P, hp, ds(col, TW)],
                                start=True, stop=True, tile_position=(HD, HD))
                            nc.any.tensor_copy(attnT[:, hp, ds(col, TW)],
                                               ps[:])

            # ------------- gate + output projection + residual -------
            with nc.named_scope("out"):
                for i in range(NT):
                    for jw in range(NJW):
                        psg = ps384.tile([P, JW], F32, tag="ps384")
                        for k in range(KT):
                            nc.tensor.matmul(
                                psg[:], xnT[:, k, ts(i, P)],
                                wT["gate_w"][:, k, ts(jw, JW)],
                                start=(k == 0), stop=(k == KT - 1))
                        gt = rwk.tile([P, JW], F32, tag="gtmp")
                        nc.vector.tensor_add(
                            gt[:], psg[:], bc["gate_b"][:, ts(jw, JW)])
                        g16 = rwk.tile([P, JW], F16, tag="g16")
                        nc.scalar.activation(g16[:], gt[:], AF.Sigmoid)

                        ps = ps384.tile([P, JW], F32, tag="ps384")
                        for k in range(KT):
                            nc.tensor.matmul(
                                ps[:], attnT[:, k, ts(i, P)],
                                wT["out_w"][:, k, ts(jw, JW)],
                                start=(k == 0), stop=(k == KT - 1))
                        ao = rwk.tile([P, JW], F32, tag="aotmp")
                        nc.vector.tensor_add(
                            ao[:], ps[:], bc["out_b"][:, ts(jw, JW)])
                        nc.any.tensor_mul(ao[:], ao[:], g16[:])
                        xr = wk.tile([P, JW], F32, tag="xres")
                        nc.sync.dma_start(
                            xr[:], x_flat[ts(i, P), ds(jw * JW, JW)])
                        ot = wk.tile([P, JW], F32, tag="otile")
                        nc.any.tensor_add(ot[:], ao[:], xr[:])
                        nc.sync.dma_start(
                            out_flat[ts(i, P), ds(jw * JW, JW)], ot[:])

    nc.finalize()
    return nc


_NC = None


def _get_nc():
    global _NC
    if _NC is None:
        _NC = build_nc()
    return _NC


def kernel(**inputs):
    nc = _get_nc()
    x = np.ascontiguousarray(inputs["x"], dtype=np.float32)
    shared = {}
    for nm in ["rope_emb", "ln_w", "ln_b", "enc_b", "qk_b", "v_b", "out_b",
               "gate_b"] + W_NAMES:
        shared[nm] = np.ascontiguousarray(inputs[nm], dtype=np.float32)
    in_maps = []
    n_cores = 8
    for c in range(n_cores):
        m = dict(shared)
        m["x"] = np.ascontiguousarray(x[c * B_LOC:(c + 1) * B_LOC])
        in_maps.append(m)
    res = bass_utils.run_bass_kernel_spmd(
        nc, in_maps, core_ids=list(range(n_cores)))
    return np.concatenate([r["out"] for r in res.results], axis=0)


# revision 9
# speedup vs baseline: 1.0270x; 1.0270x over previous
"""Trainium2 Bass kernel for nn_BDHBlock (pre-LN latent block with
softmax-free attention and sigmoid gating).

Sharding: data-parallel over batch B=16 across 8 cores (2 per core).
No collectives; outputs are concatenated on the host.

Per-core math (B_loc=2, N=1024, D=768, H=12, HD=64), all matmuls fp16
with fp32 PSUM accumulation:
  xn   = LayerNorm(x) * ln_w + ln_b              (token-major, fp32)
  lat  = relu(xn @ enc_w.T + enc_b)              (feature-major)
  qk   = rope(lat @ qk_w.T + qk_b) / sqrt(sqrt(HD))   (token-major)
  v    = lat @ v_w.T + v_b                       (token-major)
  T_h  = qk_h^T @ v_h         per (b,h)          [HD, HD]
  attn_h = qk_h @ T_h      (== (qk qk^T/8) v by associativity)
  out  = x + sigmoid(xn @ gate_w.T + gate_b) * (attn @ out_w.T + out_b)

The softmax-free attention makes scores@v associative, so the N x N
score matrices are never materialized.
"""

import os
import sys

for _p in ("/opt/trn_rl_repo", "/root/.axon_site/_ro/trn_rl_repo"):
    if os.path.isdir(_p) and _p not in sys.path:
        sys.path.insert(0, _p)

import math
import numpy as np

import concourse.bass as bass
import concourse.mybir as mybir
from concourse import bacc
from concourse import bass_utils
from concourse.bass import ts, ds
from concourse.tile import TileContext

F32 = mybir.dt.float32
F16 = mybir.dt.float16
AF = mybir.ActivationFunctionType

P = 128          # partitions
D = 768
KT = D // P      # 6 d-tiles
B_LOC = 2        # batch elements per core
SEQ = 1024
T = B_LOC * SEQ  # 2048 tokens per core
NT = T // P      # 16 token tiles
TPB = SEQ // P   # 8 token tiles per batch element
TW = 512         # token window (feature-major matmul free dim)
NTW = T // TW    # 4
JW = 384         # feature window (token-major matmul free dim)
NJW = D // JW    # 2
H = 12
HD = 64
EPS = 1e-5
QK_SCALE = 1.0 / math.sqrt(math.sqrt(HD))  # applied twice => 1/sqrt(HD)

# weight prep order: gate_w reuses enc_w's slot (enc phase is done by then)
W_NAMES = ["enc_w", "qk_w", "v_w", "out_w", "gate_w"]


def _trig_coefs():
    """Power-series coefficients for sin(x)=x*S(x^2), cos(x)=C(x^2) on
    |x|<=8 (the ACT Sin LUT is unusable outside a small range)."""
    xs = np.linspace(1e-8, 8.0, 40001)
    u = xs ** 2
    cheb = np.polynomial.chebyshev
    s = cheb.cheb2poly(cheb.chebfit(u, np.sin(xs) / xs, 12))
    c = cheb.cheb2poly(cheb.chebfit(u, np.cos(xs), 12))
    return [float(v) for v in s], [float(v) for v in c]


SIN_COEF, COS_COEF = _trig_coefs()


def build_nc():
    nc = bacc.Bacc("TRN2", target_bir_lowering=False, debug=False)

    x_in = nc.dram_tensor("x", [B_LOC, SEQ, D], F32, kind="ExternalInput")
    rope_in = nc.dram_tensor("rope_emb", [SEQ, HD], F32, kind="ExternalInput")
    vecs = {}
    for nm in ["ln_w", "ln_b", "enc_b", "qk_b", "v_b", "out_b", "gate_b"]:
        vecs[nm] = nc.dram_tensor(nm, [D], F32, kind="ExternalInput")
    w_in = {nm: nc.dram_tensor(nm, [D, D], F32, kind="ExternalInput")
            for nm in W_NAMES}
    out_t = nc.dram_tensor("out", [B_LOC, SEQ, D], F32, kind="ExternalOutput")

    x_flat = x_in.ap().rearrange("b n d -> (b n) d")
    out_flat = out_t.ap().rearrange("b n d -> (b n) d")

    with TileContext(nc) as tc:
        with (
            tc.tile_pool(name="consts", bufs=1) as cp,
            tc.tile_pool(name="wrot", bufs=4) as wrot,
            tc.tile_pool(name="big", bufs=3) as bigp,
            tc.tile_pool(name="work", bufs=2) as wk,
            tc.tile_pool(name="stats", bufs=4) as stp,
            tc.tile_pool(name="ropewk", bufs=2) as rwk,
            tc.tile_pool(name="tbuf", bufs=12) as tbp,
            tc.tile_pool(name="dram", bufs=2, space="DRAM") as dp,
            tc.tile_pool(name="dram1", bufs=1, space="DRAM") as dp1,
            tc.tile_pool(name="ps512", bufs=3, space="PSUM") as ps512,
            tc.tile_pool(name="ps384", bufs=3, space="PSUM") as ps384,
            tc.tile_pool(name="psT", bufs=2, space="PSUM") as psT,
        ):
            # ---------------- constants / weight prep ----------------
            with nc.named_scope("prep"):
                # rope tables: [128, TPB, 4, 32] = cosE, sinE, sinO, cosO
                rp = wk.tile([P, TPB, HD], F32, tag="ropein")
                nc.sync.dma_start(
                    rp[:], rope_in.ap().rearrange("(t p) d -> p t d", p=P))
                tabs = cp.tile([P, TPB, 4, HD // 2], F32, tag="ropetabs")
                eps_t = cp.tile([P, 1], F32, tag="epsc")
                nc.vector.memset(eps_t[:], EPS)
                # sin/cos via fp32 Horner on DVE (ACT Sin LUT is inaccurate
                # for |x| beyond ~pi/2)
                u = wk.tile([P, TPB, HD], F32, tag="ropeu")
                nc.vector.tensor_mul(u[:], rp[:], rp[:])

                def horner(coef, out):
                    nc.vector.tensor_scalar(
                        out[:], u[:], coef[-1], coef[-2],
                        op0=mybir.AluOpType.mult, op1=mybir.AluOpType.add)
                    for cf in coef[-3::-1]:
                        nc.vector.tensor_mul(out[:], out[:], u[:])
                        nc.vector.tensor_scalar_add(out[:], out[:], cf)

                sin_a = wk.tile([P, TPB, HD], F32, tag="ropesin")
                cos_a = wk.tile([P, TPB, HD], F32, tag="ropecos")
                horner(SIN_COEF, sin_a)
                nc.vector.tensor_mul(sin_a[:], sin_a[:], rp[:])
                horner(COS_COEF, cos_a)
                nc.vector.tensor_scalar_mul(
                    tabs[:, :, 0, :], cos_a[:, :, 0::2], QK_SCALE)
                nc.vector.tensor_scalar_mul(
                    tabs[:, :, 1, :], sin_a[:, :, 0::2], QK_SCALE)
                nc.vector.tensor_scalar_mul(
                    tabs[:, :, 2, :], sin_a[:, :, 1::2], QK_SCALE)
                nc.vector.tensor_scalar_mul(
                    tabs[:, :, 3, :], cos_a[:, :, 1::2], QK_SCALE)

                # broadcast-to-all-partitions tiles for free-dim vectors
                bc = {}
                for nm in ["ln_w", "ln_b", "qk_b", "v_b", "out_b", "gate_b"]:
                    bc[nm] = cp.tile([P, D], F16, tag=f"bc_{nm}",
                                     name=f"bc_{nm}")
                    nc.gpsimd.dma_start(
                        out=bc[nm][:],
                        in_=vecs[nm].ap()[None, :].to_broadcast((P, D)))
                # enc bias, per-partition layout [128, KT]
                encb = cp.tile([P, KT], F32, tag="encb")
                nc.sync.dma_start(
                    encb[:], vecs["enc_b"].ap().rearrange("(k p) -> p k", p=P))

                # weights: load fp32, cast fp16, roundtrip through DRAM with
                # a transposing read -> W^T laid out [d(part), k, j]
                wT = {}
                for nm in W_NAMES:
                    wT[nm] = wrot.tile([P, KT, D], F16, tag="wT",
                                       name=f"wT_{nm}")
                    w16d = dp.tile([D, D], F16, tag="wdram", name=f"w16_{nm}")
                    for k in range(KT):
                        wld = wk.tile([P, D], F32, tag="wload")
                        nc.sync.dma_start(wld[:], w_in[nm].ap()[ts(k, P), :])
                        wc = wk.tile([P, D], F16, tag="wcast")
                        nc.any.tensor_copy(wc[:], wld[:])
                        nc.sync.dma_start(w16d[ts(k, P), :], wc[:])
                    for k in range(KT):
                        nc.sync.dma_start(wT[nm][:, k, :], w16d[:, ts(k, P)],
                                          transpose=True)

            xn_dram = dp1.tile([T, D], F16, tag="xn_dram")
            qk_dram = dp1.tile([T, D], F16, tag="qk_dram")

            # ---------------- LayerNorm (token-major) ----------------
            with nc.named_scope("ln"):
                for i in range(NT):
                    xt = wk.tile([P, D], F32, tag="xin")
                    nc.sync.dma_start(xt[:], x_flat[ts(i, P), :])
                    xg = xt[:].rearrange("p (s c) -> p s c", c=256)
                    stats = stp.tile([P, 3, 6], F32, tag="bnstats")
                    for s in range(3):
                        nc.vector.bn_stats(stats[:, s, :], xg[:, s, :])
                    mv = stp.tile([P, 2], F32, tag="bnmv")
                    nc.vector.bn_aggr(mv[:], stats[:])
                    rs = stp.tile([P, 1], F32, tag="rstd")
                    nc.scalar.activation(rs[:], mv[:, 1:2], AF.Sqrt,
                                         bias=eps_t[:])
                    nc.vector.reciprocal(rs[:], rs[:])
                    nc.vector.tensor_scalar(
                        xt[:], xt[:], mv[:, 0:1], rs[:],
                        op0=mybir.AluOpType.subtract, op1=mybir.AluOpType.mult)
                    nc.any.tensor_mul(xt[:], xt[:], bc["ln_w"][:])
                    xn16 = wk.tile([P, D], F16, tag="xn16")
                    nc.any.tensor_add(xn16[:], xt[:], bc["ln_b"][:])
                    nc.sync.dma_start(xn_dram[ts(i, P), :], xn16[:])

            # xn^T: feature-major [128, KT, T]; lives until the gate matmuls
            # at the very end, so it gets its own slot outside the rotation.
            xnT = cp.tile([P, KT, T], F16, tag="xnT")
            with nc.named_scope("xnT"):
                for k in range(KT):
                    for h2 in range(2):
                        nc.sync.dma_start(
                            xnT[:, k, ts(h2, SEQ)],
                            xn_dram[ts(h2, SEQ), ts(k, P)], transpose=True)

            # ---------------- encoder: latT = relu(Wenc @ xn^T) ------
            latT = bigp.tile([P, KT, T], F16, tag="big", name="latT")
            with nc.named_scope("enc"):
                for tw in range(NTW):
                    for j in range(KT):
                        ps = ps512.tile([P, TW], F32, tag="ps512")
                        for k in range(KT):
                            nc.tensor.matmul(
                                ps[:], wT["enc_w"][:, k, ts(j, P)],
                                xnT[:, k, ts(tw, TW)],
                                start=(k == 0), stop=(k == KT - 1))
                        nc.scalar.activation(latT[:, j, ts(tw, TW)], ps[:],
                                             AF.Relu, bias=encb[:, j:j + 1])

            # ---------------- qk (token-major) + rope ----------------
            qkR = bigp.tile([P, NT, D], F16, tag="big", name="qkR")
            with nc.named_scope("qk"):
                for i in range(NT):
                    ti = i % TPB
                    for jw in range(NJW):
                        ps = ps384.tile([P, JW], F32, tag="ps384")
                        for k in range(KT):
                            nc.tensor.matmul(
                                ps[:], latT[:, k, ts(i, P)],
                                wT["qk_w"][:, k, ts(jw, JW)],
                                start=(k == 0), stop=(k == KT - 1))
                        xb = rwk.tile([P, JW], F32, tag="ropexb")
                        nc.vector.tensor_add(
                            xb[:], ps[:], bc["qk_b"][:, ts(jw, JW)])
                        # rope on 6 heads at once via step-0 broadcast tables
                        xbh = xb[:].rearrange("p (h d) -> p h d", d=HD)
                        x1 = xbh[:, :, 0::2]
                        x2 = xbh[:, :, 1::2]
                        o = qkR[:, i, ts(jw, JW)].rearrange(
                            "p (h d) -> p h d", d=HD)
                        nh = JW // HD
                        cosE = tabs[:, ti, 0, None, :].to_broadcast(
                            (P, nh, HD // 2))
                        sinE = tabs[:, ti, 1, None, :].to_broadcast(
                            (P, nh, HD // 2))
                        sinO = tabs[:, ti, 2, None, :].to_broadcast(
                            (P, nh, HD // 2))
                        cosO = tabs[:, ti, 3, None, :].to_broadcast(
                            (P, nh, HD // 2))
                        p1 = rwk.tile([P, nh, HD // 2], F32, tag="ropep1")
                        p2 = rwk.tile([P, nh, HD // 2], F32, tag="ropep2")
                        nc.any.tensor_mul(p1[:], x1, cosE)
                        nc.any.tensor_mul(p2[:], x2, sinE)
                        nc.any.tensor_sub(o[:, :, 0:HD // 2], p1[:], p2[:])
                        p3 = rwk.tile([P, nh, HD // 2], F32, tag="ropep1")
                        p4 = rwk.tile([P, nh, HD // 2], F32, tag="ropep2")
                        nc.any.tensor_mul(p3[:], x1, sinO)
                        nc.any.tensor_mul(p4[:], x2, cosO)
                        nc.any.tensor_add(o[:, :, HD // 2:], p3[:], p4[:])
                    nc.sync.dma_start(qk_dram[ts(i, P), :], qkR[:, i, :])

            # ---------------- v (token-major) ------------------------
            vtm = bigp.tile([P, NT, D], F16, tag="big", name="v")
            with nc.named_scope("v"):
                for i in range(NT):
                    for jw in range(NJW):
                        ps = ps384.tile([P, JW], F32, tag="ps384")
                        for k in range(KT):
                            nc.tensor.matmul(
                                ps[:], latT[:, k, ts(i, P)],
                                wT["v_w"][:, k, ts(jw, JW)],
                                start=(k == 0), stop=(k == KT - 1))
                        nc.vector.tensor_add(
                            vtm[:, i, ts(jw, JW)], ps[:],
                            bc["v_b"][:, ts(jw, JW)])

            # qk^T: feature-major [128, KT, T] (takes latT's slot)
            qkT = bigp.tile([P, KT, T], F16, tag="big", name="qkT")
            with nc.named_scope("qkT"):
                for k in range(KT):
                    for h2 in range(2):
                        nc.sync.dma_start(
                            qkT[:, k, ts(h2, SEQ)],
                            qk_dram[ts(h2, SEQ), ts(k, P)], transpose=True)

            # ---------------- attention ------------------------------
            # M1: T_h = qk_h^T @ v_h  [HD, HD] per (b, head); head pairs
            # packed into array column halves.  M2: attnT_h = T_h^T @ qkT_h.
            # All M1 products first so qkR/v are fully released before the
            # attnT slot (which reuses qkR's ring slot) is first written.
            t16s = {}
            with nc.named_scope("attn_m1"):
                for b in range(B_LOC):
                    for hp in range(KT):
                        hA, hB = 2 * hp, 2 * hp + 1
                        pt = psT.tile([P, HD], F32, tag="psT")
                        for m in range(TPB):
                            mt = b * TPB + m
                            nc.tensor.matmul(
                                pt[0:HD, :],
                                qkR[:, mt, ts(hA, HD)], vtm[:, mt, ts(hA, HD)],
                                start=(m == 0), stop=(m == TPB - 1),
                                tile_position=(0, 0))
                            nc.tensor.matmul(
                                pt[HD:P, :],
                                qkR[:, mt, ts(hB, HD)], vtm[:, mt, ts(hB, HD)],
                                start=(m == 0), stop=(m == TPB - 1),
                                tile_position=(0, HD))
                        t16 = tbp.tile([P, HD], F16, tag="t16",
                                       name=f"t16_{b}_{hp}")
                        nc.any.tensor_copy(t16[:], pt[:])
                        t16s[(b, hp)] = t16

            attnT = bigp.tile([P, KT, T], F16, tag="big", name="attnT")
            with nc.named_scope("attn_m2"):
                for b in range(B_LOC):
                    for hp in range(KT):
                        t16 = t16s[(b, hp)]
                        for nw in range(2):
                            col = b * SEQ + nw * TW
                            ps = ps512.tile([P, TW], F32, tag="ps512")
                            nc.tensor.matmul(
                                ps[0:HD, :], t16[0:HD, :],
                                qkT[0:HD, hp, ds(col, TW)],
                                start=True, stop=True, tile_position=(0, 0))
                            nc.tensor.matmul(
                                ps[HD:P, :], t16[HD:P, :],
                                qkT[HD:P, hp, ds(col, TW)],
                                start=True, stop=True, tile_position=(HD, HD))
                            nc.any.tensor_copy(attnT[:, hp, ds(col, TW)],
                                               ps[:])

            # ------------- gate + output projection + residual -------
            with nc.named_scope("out"):
                for i in range(NT):
                    for jw in range(NJW):
                        psg = ps384.tile([P, JW], F32, tag="ps384")
                        for k in range(KT):
                            nc.tensor.matmul(
                                psg[:], xnT[:, k, ts(i, P)],
                                wT["gate_w"][:, k, ts(jw, JW)],
                                start=(k == 0), stop=(k == KT - 1))
                        gt = rwk.tile([P, JW], F32, tag="gtmp")
                        nc.vector.tensor_add(
                            gt[:], psg[:], bc["gate_b"][:, ts(jw, JW)])
                        g16 = rwk.tile([P, JW], F16, tag="g16")
                        nc.scalar.activation(g16[:], gt[:], AF.Sigmoid)

                        ps = ps384.tile([P, JW], F32, tag="ps384")
                        for k in range(KT):
                            nc.tensor.matmul(
                                ps[:], attnT[:, k, ts(i, P)],
                                wT["out_w"][:, k, ts(jw, JW)],
                                start=(k == 0), stop=(k == KT - 1))
                        ao = rwk.tile([P, JW], F32, tag="aotmp")
                        nc.vector.tensor_add(
                            ao[:], ps[:], bc["out_b"][:, ts(jw, JW)])
                        nc.any.tensor_mul(ao[:], ao[:], g16[:])
                        xr = wk.tile([P, JW], F32, tag="xres")
                        nc.sync.dma_start(
                            xr[:], x_flat[ts(i, P), ds(jw * JW, JW)])
                        ot = wk.tile([P, JW], F32, tag="otile")
                        nc.any.tensor_add(ot[:], ao[:], xr[:])
                        nc.sync.dma_start(
                            out_flat[ts(i, P), ds(jw * JW, JW)], ot[:])

    nc.finalize()
    return nc


_NC = None


def _get_nc():
    global _NC
    if _NC is None:
        _NC = build_nc()
    return _NC


def kernel(**inputs):
    nc = _get_nc()
    x = np.ascontiguousarray(inputs["x"], dtype=np.float32)
    shared = {}
    for nm in ["rope_emb", "ln_w", "ln_b", "enc_b", "qk_b", "v_b", "out_b",
               "gate_b"] + W_NAMES:
        shared[nm] = np.ascontiguousarray(inputs[nm], dtype=np.float32)
    in_maps = []
    n_cores = 8
    for c in range(n_cores):
        m = dict(shared)
        m["x"] = np.ascontiguousarray(x[c * B_LOC:(c + 1) * B_LOC])
        in_maps.append(m)
    res = bass_utils.run_bass_kernel_spmd(
        nc, in_maps, core_ids=list(range(n_cores)))
    return np.concatenate([r["out"] for r in res.results], axis=0)


# revision 11
# speedup vs baseline: 1.2524x; 1.2195x over previous
"""Trainium2 Bass kernel for nn_BDHBlock (pre-LN latent block with
softmax-free attention and sigmoid gating).

Sharding: data-parallel over batch B=16 across 8 cores (2 per core).
No collectives; outputs are concatenated on the host.

Per-core math (B_loc=2, N=1024, D=768, H=12, HD=64), all matmuls fp16
with fp32 PSUM accumulation:
  xn   = LayerNorm(x) * ln_w + ln_b              (token-major, fp32)
  lat  = relu(xn @ enc_w.T + enc_b)              (feature-major)
  qk   = rope(lat @ qk_w.T + qk_b) / sqrt(sqrt(HD))   (token-major)
  v    = lat @ v_w.T + v_b                       (token-major)
  T_h  = qk_h^T @ v_h         per (b,h)          [HD, HD]
  attn_h = qk_h @ T_h      (== (qk qk^T/8) v by associativity)
  out  = x + sigmoid(xn @ gate_w.T + gate_b) * (attn @ out_w.T + out_b)

The softmax-free attention makes scores@v associative, so the N x N
score matrices are never materialized.
"""

import os
import sys

for _p in ("/opt/trn_rl_repo", "/root/.axon_site/_ro/trn_rl_repo"):
    if os.path.isdir(_p) and _p not in sys.path:
        sys.path.insert(0, _p)

import math
import numpy as np

import concourse.bass as bass
import concourse.mybir as mybir
from concourse import bacc
from concourse import bass_utils
from concourse.bass import ts, ds
from concourse.tile import TileContext
from concourse.masks import make_identity

F32 = mybir.dt.float32
F16 = mybir.dt.float16
AF = mybir.ActivationFunctionType

P = 128          # partitions
D = 768
KT = D // P      # 6 d-tiles
B_LOC = 2        # batch elements per core
SEQ = 1024
T = B_LOC * SEQ  # 2048 tokens per core
NT = T // P      # 16 token tiles
TPB = SEQ // P   # 8 token tiles per batch element
TW = 512         # token window (feature-major matmul free dim)
NTW = T // TW    # 4
JW = 384         # feature window (token-major matmul free dim)
NJW = D // JW    # 2
H = 12
HD = 64
EPS = 1e-5
QK_SCALE = 1.0 / math.sqrt(math.sqrt(HD))  # applied twice => 1/sqrt(HD)

# weight prep order: gate_w reuses enc_w's slot (enc phase is done by then)
W_NAMES = ["enc_w", "qk_w", "v_w", "out_w", "gate_w"]


def _trig_coefs():
    """Power-series coefficients for sin(x)=x*S(x^2), cos(x)=C(x^2) on
    |x|<=8 (the ACT Sin LUT is unusable outside a small range)."""
    xs = np.linspace(1e-8, 8.0, 40001)
    u = xs ** 2
    cheb = np.polynomial.chebyshev
    s = cheb.cheb2poly(cheb.chebfit(u, np.sin(xs) / xs, 12))
    c = cheb.cheb2poly(cheb.chebfit(u, np.cos(xs), 12))
    return [float(v) for v in s], [float(v) for v in c]


SIN_COEF, COS_COEF = _trig_coefs()


def build_nc():
    nc = bacc.Bacc("TRN2", target_bir_lowering=False, debug=False)

    x_in = nc.dram_tensor("x", [B_LOC, SEQ, D], F32, kind="ExternalInput")
    rope_in = nc.dram_tensor("rope_emb", [SEQ, HD], F32, kind="ExternalInput")
    vecs = {}
    for nm in ["ln_w", "ln_b", "enc_b", "qk_b", "v_b", "out_b", "gate_b"]:
        vecs[nm] = nc.dram_tensor(nm, [D], F32, kind="ExternalInput")
    w_in = {nm: nc.dram_tensor(nm, [D, D], F32, kind="ExternalInput")
            for nm in W_NAMES}
    out_t = nc.dram_tensor("out", [B_LOC, SEQ, D], F32, kind="ExternalOutput")

    x_flat = x_in.ap().rearrange("b n d -> (b n) d")
    out_flat = out_t.ap().rearrange("b n d -> (b n) d")

    with TileContext(nc) as tc:
        with (
            tc.tile_pool(name="consts", bufs=1) as cp,
            tc.tile_pool(name="wrot", bufs=4) as wrot,
            tc.tile_pool(name="big", bufs=3) as bigp,
            tc.tile_pool(name="work", bufs=2) as wk,
            tc.tile_pool(name="stats", bufs=4) as stp,
            tc.tile_pool(name="ropewk", bufs=2) as rwk,
            tc.tile_pool(name="tbuf", bufs=12) as tbp,
            tc.tile_pool(name="dram1", bufs=1, space="DRAM") as dp1,
            tc.tile_pool(name="ps512", bufs=3, space="PSUM") as ps512,
            tc.tile_pool(name="ps384", bufs=2, space="PSUM") as ps384,
            tc.tile_pool(name="psW", bufs=1, space="PSUM") as psW,
            tc.tile_pool(name="psX", bufs=1, space="PSUM") as psX,
            tc.tile_pool(name="psT", bufs=1, space="PSUM") as psT,
        ):
            # ---------------- constants / weight prep ----------------
            with nc.named_scope("prep"):
                # rope tables: [128, TPB, 4, 32] = cosE, sinE, sinO, cosO
                rp = wk.tile([P, TPB, HD], F32, tag="ropein")
                nc.sync.dma_start(
                    rp[:], rope_in.ap().rearrange("(t p) d -> p t d", p=P))
                tabs = cp.tile([P, TPB, 4, HD // 2], F32, tag="ropetabs")
                eps_t = cp.tile([P, 1], F32, tag="epsc")
                nc.vector.memset(eps_t[:], EPS)
                # sin/cos via fp32 Horner on DVE (ACT Sin LUT is inaccurate
                # for |x| beyond ~pi/2)
                u = wk.tile([P, TPB, HD], F32, tag="ropeu")
                nc.vector.tensor_mul(u[:], rp[:], rp[:])

                def horner(coef, out):
                    nc.vector.tensor_scalar(
                        out[:], u[:], coef[-1], coef[-2],
                        op0=mybir.AluOpType.mult, op1=mybir.AluOpType.add)
                    for cf in coef[-3::-1]:
                        nc.vector.tensor_mul(out[:], out[:], u[:])
                        nc.vector.tensor_scalar_add(out[:], out[:], cf)

                sin_a = wk.tile([P, TPB, HD], F32, tag="ropesin")
                cos_a = wk.tile([P, TPB, HD], F32, tag="ropecos")
                horner(SIN_COEF, sin_a)
                nc.vector.tensor_mul(sin_a[:], sin_a[:], rp[:])
                horner(COS_COEF, cos_a)
                nc.vector.tensor_scalar_mul(
                    tabs[:, :, 0, :], cos_a[:, :, 0::2], QK_SCALE)
                nc.vector.tensor_scalar_mul(
                    tabs[:, :, 1, :], sin_a[:, :, 0::2], QK_SCALE)
                nc.vector.tensor_scalar_mul(
                    tabs[:, :, 2, :], sin_a[:, :, 1::2], QK_SCALE)
                nc.vector.tensor_scalar_mul(
                    tabs[:, :, 3, :], cos_a[:, :, 1::2], QK_SCALE)

                # broadcast-to-all-partitions tiles for free-dim vectors
                bc = {}
                for nm in ["ln_w", "ln_b", "qk_b", "v_b", "out_b", "gate_b"]:
                    bc[nm] = cp.tile([P, D], F16, tag=f"bc_{nm}",
                                     name=f"bc_{nm}")
                    nc.gpsimd.dma_start(
                        out=bc[nm][:],
                        in_=vecs[nm].ap()[None, :].to_broadcast((P, D)))
                # enc bias, per-partition layout [128, KT]
                encb = cp.tile([P, KT], F32, tag="encb")
                nc.sync.dma_start(
                    encb[:], vecs["enc_b"].ap().rearrange("(k p) -> p k", p=P))

                # identity for PE-mode transposes
                ident = cp.tile([P, P], F16, tag="ident")
                make_identity(nc, ident[:])

                # weights: load fp32, cast fp16, transpose 128x128 blocks on
                # the PE -> W^T laid out [d(part), k, j]
                wT = {}
                for nm in W_NAMES:
                    wT[nm] = wrot.tile([P, KT, D], F16, tag="wT",
                                       name=f"wT_{nm}")
                    for k in range(KT):
                        wld = wk.tile([P, D], F32, tag="wload")
                        nc.sync.dma_start(wld[:], w_in[nm].ap()[ts(k, P), :])
                        wc = wk.tile([P, D], F16, tag="wcast")
                        nc.any.tensor_copy(wc[:], wld[:])
                        for kd in range(KT):
                            ptr = psW.tile([P, P], F16, tag="psW",
                                           name=f"ptr_{nm}_{k}_{kd}")
                            nc.tensor.transpose(ptr[:], wc[:, ts(kd, P)],
                                                ident[:])
                            nc.any.tensor_copy(wT[nm][:, kd, ts(k, P)],
                                               ptr[:])

            qk_dram = dp1.tile([T, D], F16, tag="qk_dram")

            # xn^T: feature-major [128, KT, T]; lives until the gate matmuls
            # at the very end, so it gets its own slot outside the rotation.
            xnT = cp.tile([P, KT, T], F16, tag="xnT")

            # ---------------- LayerNorm (token-major) ----------------
            with nc.named_scope("ln"):
                for i in range(NT):
                    xt = wk.tile([P, D], F32, tag="xin")
                    nc.sync.dma_start(xt[:], x_flat[ts(i, P), :])
                    xg = xt[:].rearrange("p (s c) -> p s c", c=256)
                    stats = stp.tile([P, 3, 6], F32, tag="bnstats")
                    for s in range(3):
                        nc.vector.bn_stats(stats[:, s, :], xg[:, s, :])
                    mv = stp.tile([P, 2], F32, tag="bnmv")
                    nc.vector.bn_aggr(mv[:], stats[:])
                    rs = stp.tile([P, 1], F32, tag="rstd")
                    nc.scalar.activation(rs[:], mv[:, 1:2], AF.Sqrt,
                                         bias=eps_t[:])
                    nc.vector.reciprocal(rs[:], rs[:])
                    nc.vector.tensor_scalar(
                        xt[:], xt[:], mv[:, 0:1], rs[:],
                        op0=mybir.AluOpType.subtract, op1=mybir.AluOpType.mult)
                    nc.any.tensor_mul(xt[:], xt[:], bc["ln_w"][:])
                    xn16 = wk.tile([P, D], F16, tag="xn16")
                    nc.any.tensor_add(xn16[:], xt[:], bc["ln_b"][:])
                    for k in range(KT):
                        ptr = psX.tile([P, P], F16, tag="psX",
                                       name=f"ptr_xn_{i}_{k}")
                        nc.tensor.transpose(ptr[:], xn16[:, ts(k, P)],
                                            ident[:])
                        nc.any.tensor_copy(xnT[:, k, ts(i, P)], ptr[:])

            # ---------------- encoder: latT = relu(Wenc @ xn^T) ------
            latT = bigp.tile([P, KT, T], F16, tag="big", name="latT")
            with nc.named_scope("enc"):
                for tw in range(NTW):
                    for j in range(KT):
                        ps = ps512.tile([P, TW], F32, tag="ps512")
                        for k in range(KT):
                            nc.tensor.matmul(
                                ps[:], wT["enc_w"][:, k, ts(j, P)],
                                xnT[:, k, ts(tw, TW)],
                                start=(k == 0), stop=(k == KT - 1))
                        nc.scalar.activation(latT[:, j, ts(tw, TW)], ps[:],
                                             AF.Relu, bias=encb[:, j:j + 1])

            # ---------------- qk (token-major) + rope ----------------
            qkR = bigp.tile([P, NT, D], F16, tag="big", name="qkR")
            with nc.named_scope("qk"):
                for i in range(NT):
                    ti = i % TPB
                    for jw in range(NJW):
                        ps = ps384.tile([P, JW], F32, tag="ps384")
                        for k in range(KT):
                            nc.tensor.matmul(
                                ps[:], latT[:, k, ts(i, P)],
                                wT["qk_w"][:, k, ts(jw, JW)],
                                start=(k == 0), stop=(k == KT - 1))
                        xb = rwk.tile([P, JW], F32, tag="ropexb")
                        nc.vector.tensor_add(
                            xb[:], ps[:], bc["qk_b"][:, ts(jw, JW)])
                        # rope on 6 heads at once via step-0 broadcast tables
                        xbh = xb[:].rearrange("p (h d) -> p h d", d=HD)
                        x1 = xbh[:, :, 0::2]
                        x2 = xbh[:, :, 1::2]
                        o = qkR[:, i, ts(jw, JW)].rearrange(
                            "p (h d) -> p h d", d=HD)
                        nh = JW // HD
                        cosE = tabs[:, ti, 0, None, :].to_broadcast(
                            (P, nh, HD // 2))
                        sinE = tabs[:, ti, 1, None, :].to_broadcast(
                            (P, nh, HD // 2))
                        sinO = tabs[:, ti, 2, None, :].to_broadcast(
                            (P, nh, HD // 2))
                        cosO = tabs[:, ti, 3, None, :].to_broadcast(
                            (P, nh, HD // 2))
                        p1 = rwk.tile([P, nh, HD // 2], F32, tag="ropep1")
                        p2 = rwk.tile([P, nh, HD // 2], F32, tag="ropep2")
                        nc.any.tensor_mul(p1[:], x1, cosE)
                        nc.gpsimd.tensor_mul(p2[:], x2, sinE)
                        nc.any.tensor_sub(o[:, :, 0:HD // 2], p1[:], p2[:])
                        p3 = rwk.tile([P, nh, HD // 2], F32, tag="ropep1")
                        p4 = rwk.tile([P, nh, HD // 2], F32, tag="ropep2")
                        nc.any.tensor_mul(p3[:], x1, sinO)
                        nc.gpsimd.tensor_mul(p4[:], x2, cosO)
                        nc.any.tensor_add(o[:, :, HD // 2:], p3[:], p4[:])
                    nc.sync.dma_start(qk_dram[ts(i, P), :], qkR[:, i, :])

            # ---------------- v (token-major) ------------------------
            vtm = bigp.tile([P, NT, D], F16, tag="big", name="v")
            with nc.named_scope("v"):
                for i in range(NT):
                    for jw in range(NJW):
                        ps = ps384.tile([P, JW], F32, tag="ps384")
                        for k in range(KT):
                            nc.tensor.matmul(
                                ps[:], latT[:, k, ts(i, P)],
                                wT["v_w"][:, k, ts(jw, JW)],
                                start=(k == 0), stop=(k == KT - 1))
                        nc.vector.tensor_add(
                            vtm[:, i, ts(jw, JW)], ps[:],
                            bc["v_b"][:, ts(jw, JW)])

            # qk^T: feature-major [128, KT, T] (takes latT's slot)
            qkT = bigp.tile([P, KT, T], F16, tag="big", name="qkT")
            with nc.named_scope("qkT"):
                for k in range(KT):
                    for h2 in range(2):
                        nc.sync.dma_start(
                            qkT[:, k, ts(h2, SEQ)],
                            qk_dram[ts(h2, SEQ), ts(k, P)], transpose=True)

            # ---------------- attention ------------------------------
            # M1: T_h = qk_h^T @ v_h  [HD, HD] per (b, head); head pairs
            # packed into array column halves.  M2: attnT_h = T_h^T @ qkT_h.
            # All M1 products first so qkR/v are fully released before the
            # attnT slot (which reuses qkR's ring slot) is first written.
            t16s = {}
            with nc.named_scope("attn_m1"):
                for b in range(B_LOC):
                    for hp in range(KT):
                        hA, hB = 2 * hp, 2 * hp + 1
                        pt = psT.tile([P, HD], F32, tag="psT",
                                      name=f"ptm1_{b}_{hp}")
                        for m in range(TPB):
                            mt = b * TPB + m
                            nc.tensor.matmul(
                                pt[0:HD, :],
                                qkR[:, mt, ts(hA, HD)], vtm[:, mt, ts(hA, HD)],
                                start=(m == 0), stop=(m == TPB - 1),
                                tile_position=(0, 0))
                            nc.tensor.matmul(
                                pt[HD:P, :],
                                qkR[:, mt, ts(hB, HD)], vtm[:, mt, ts(hB, HD)],
                                start=(m == 0), stop=(m == TPB - 1),
                                tile_position=(0, HD))
                        t16 = tbp.tile([P, HD], F16, tag="t16",
                                       name=f"t16_{b}_{hp}")
                        nc.any.tensor_copy(t16[:], pt[:])
                        t16s[(b, hp)] = t16

            attnT = bigp.tile([P, KT, T], F16, tag="big", name="attnT")
            with nc.named_scope("attn_m2"):
                for b in range(B_LOC):
                    for hp in range(KT):
                        t16 = t16s[(b, hp)]
                        for nw in range(2):
                            col = b * SEQ + nw * TW
                            ps = ps512.tile([P, TW], F32, tag="ps512")
                            nc.tensor.matmul(
                                ps[0:HD, :], t16[0:HD, :],
                                qkT[0:HD, hp, ds(col, TW)],
                                start=True, stop=True, tile_position=(0, 0))
                            nc.tensor.matmul(
                                ps[HD:P, :], t16[HD:P, :],
                                qkT[HD:P, hp, ds(col, TW)],
                                start=True, stop=True, tile_position=(HD, HD))
                            nc.any.tensor_copy(attnT[:, hp, ds(col, TW)],
                                               ps[:])

            # ------------- gate + output projection + residual -------
            with nc.named_scope("out"):
                for i in range(NT):
                    xr = wk.tile([P, D], F32, tag="xres")
                    nc.sync.dma_start(xr[:], x_flat[ts(i, P), :])
                    ot = wk.tile([P, D], F32, tag="otile")
                    for jw in range(NJW):
                        psg = ps384.tile([P, JW], F32, tag="ps384")
                        for k in range(KT):
                            nc.tensor.matmul(
                                psg[:], xnT[:, k, ts(i, P)],
                                wT["gate_w"][:, k, ts(jw, JW)],
                                start=(k == 0), stop=(k == KT - 1))
                        gt = rwk.tile([P, JW], F32, tag="gtmp")
                        nc.vector.tensor_add(
                            gt[:], psg[:], bc["gate_b"][:, ts(jw, JW)])
                        g16 = rwk.tile([P, JW], F16, tag="g16")
                        nc.scalar.activation(g16[:], gt[:], AF.Sigmoid)

                        ps = ps384.tile([P, JW], F32, tag="ps384")
                        for k in range(KT):
                            nc.tensor.matmul(
                                ps[:], attnT[:, k, ts(i, P)],
                                wT["out_w"][:, k, ts(jw, JW)],
                                start=(k == 0), stop=(k == KT - 1))
                        ao = rwk.tile([P, JW], F32, tag="aotmp")
                        nc.vector.tensor_add(
                            ao[:], ps[:], bc["out_b"][:, ts(jw, JW)])
                        nc.any.tensor_mul(ao[:], ao[:], g16[:])
                        nc.any.tensor_add(ot[:, ds(jw * JW, JW)], ao[:],
                                          xr[:, ds(jw * JW, JW)])
                    nc.sync.dma_start(out_flat[ts(i, P), :], ot[:])

    nc.finalize()
    return nc


_NC = None


def _get_nc():
    global _NC
    if _NC is None:
        _NC = build_nc()
    return _NC


def kernel(**inputs):
    nc = _get_nc()
    x = np.ascontiguousarray(inputs["x"], dtype=np.float32)
    shared = {}
    for nm in ["rope_emb", "ln_w", "ln_b", "enc_b", "qk_b", "v_b", "out_b",
               "gate_b"] + W_NAMES:
        shared[nm] = np.ascontiguousarray(inputs[nm], dtype=np.float32)
    in_maps = []
    n_cores = 8
    for c in range(n_cores):
        m = dict(shared)
        m["x"] = np.ascontiguousarray(x[c * B_LOC:(c + 1) * B_LOC])
        in_maps.append(m)
    res = bass_utils.run_bass_kernel_spmd(
        nc, in_maps, core_ids=list(range(n_cores)))
    return np.concatenate([r["out"] for r in res.results], axis=0)
